# revision 48
# baseline (speedup 1.0000x reference)
"""Trainium2 Bass kernel for GCN-biased sparse attention (nn_Attention_37589553775245).

Reference computation (per batch b of 8, one NeuronCore each):
    qkv = x @ w_qkv; q,k,v per head (H=8, DH=64)
    attn = softmax(q k^T / sqrt(DH)) + A_hat        (A_hat = D^-1/2 (ceil(adj)+I) D^-1/2)
    out = (attn @ v) @ w_out + b_out
Sharding: pure batch-parallel across the 8 cores (B=8). A_hat / dinv computed
on host (cheap, from adj only) and replicated; weights replicated.

v3 design notes (PE-bound; v2 ran the bf16 PE floor of 95.6us, v3 drops it
to ~84.5us with fp8 DoubleRow on every matmul whose operands can carry an
fp8+residual pair without precision loss):
  - fp8-pair trick at a global 16x scale: a = a8 + ar/16 with a8=e4m3(a),
    ar8=e4m3(16(a-a8)). Computing 16*qkv = (16x8)w8 + x8 wr8 + xr8 w8 needs
    only exact exponent-shifted copies (no /16 partner tensors, no fp8
    denormal range issues), three DoubleRow passes = 0.75x the bf16 cost.
    All downstream values ride at 16x; the final /16 is folded into w_out on
    the host. exp() scale becomes SCALE/256 (scores are (16q).(16k)).
  - ahat path: A_hat@V = dinv_i*(A@(dinv_j*v)) with binary A EXACT in fp8.
    v8s = e4m3(16*dinv*v) (ACT, psum->sbuf with per-partition dinv scale),
    vp16 = 16*dinv*v f32 (Pool - otherwise idle), r8 = e4m3(vp16-v8s) (DVE).
    8 DoubleRow matmuls per (ft,c) vs 8 bf16: half the PE time; the outer
    dinv_i rides the PSUM->SBUF copy as a tensor_mul with a broadcast row.
    End-to-end error 2.8e-3 (max-norm, gate 2e-2) - better than v2's 3.4e-3
    because fp8-pair carries ~11 mantissa bits vs bf16's 8.
  - scores and attn@v stay bf16: scores are PSUM-output-rate-bound (fp8
    helps only 2x but needs 3 compensation passes = net loss) and et has no
    cheap residual (one ACT pass only). PE floor: 24576(qk)+12288(v)+65536
    (scores)+65536(attnv)+16384(ahat)+18432(outproj) = 202752cyc = 84.5us.
  - everything else follows v2: scores computed transposed per head, softmax
    denominator rides ones-columns in vaug, interleaved head pairs with the
    exp stream two batches behind, one filler PE stream, split-ft3 endgame,
    b_out added on host.
"""

import os
import sys

import numpy as np

for _p in ("/opt/trn_rl_repo", "/root/.axon_site/_ro/trn_rl_repo"):
    if _p not in sys.path and os.path.isdir(_p):
        sys.path.insert(0, _p)

import ml_dtypes  # noqa: E402

import concourse.bass as bass  # noqa: E402
import concourse.mybir as mybir  # noqa: E402
import concourse.tile as tile  # noqa: E402
from concourse import bacc  # noqa: E402
from concourse.bass_utils import run_bass_kernel_spmd  # noqa: E402

B, N, DIM, H, DH = 8, 1024, 512, 8, 64
F = H * DH          # 512, inner dim
NT = N // 128       # 8 n-tiles (also j-tiles)
DT = DIM // 128     # 4 dim-tiles
DP = DT // 2        # 2 dt-pairs (DoubleRow contraction granularity)
FT = F // 128       # 4 f-tiles
NC2 = N // 512      # 2 i-chunks of 512
SCALE = DH ** -0.5

F32 = mybir.dt.float32
F32R = mybir.dt.float32r
BF16 = mybir.dt.bfloat16
FP8 = mybir.dt.float8e4
DR = mybir.MatmulPerfMode.DoubleRow

_PROGRAM = None
_last_in_maps = None


def _build_program(reps=1, zero_vaug=None, intra0=4, intra1=3,
                   exps_bufs=10, small_bufs=6):
    if zero_vaug is None:
        zero_vaug = os.environ.get("K_ZERO_VAUG", "1") == "1"
    nc = bacc.Bacc("TRN2", target_bir_lowering=False, debug=False, num_devices=8)

    # packed inputs: one DMA covers all three x parts (or both w parts)
    # of a slice, so the startup-critical prefix is 3 HWDGE issues, not 8
    xp_d = nc.dram_tensor("xp", [3 * DIM, N], FP8, kind="ExternalInput")
    wp_d = nc.dram_tensor("wp", [2 * DIM, 3 * F], FP8, kind="ExternalInput")
    at8_d = nc.dram_tensor("at8", [N, N], FP8, kind="ExternalInput")
    wout_d = nc.dram_tensor("wout", [F, DIM], F32R, kind="ExternalInput")
    dinvp_d = nc.dram_tensor("dinvp", [128, NT], F32, kind="ExternalInput")
    dinvr_d = nc.dram_tensor("dinvr", [1, N], F32, kind="ExternalInput")
    out_d = nc.dram_tensor("out", [N, DIM], F32, kind="ExternalOutput")

    with tile.TileContext(nc) as tc:
        with (
            tc.tile_pool(name="big", bufs=1) as big,
            tc.tile_pool(name="ps_mm", bufs=2, space="PSUM") as ps_mm,
            tc.tile_pool(name="ps_s", bufs=2, space="PSUM") as ps_s,
            tc.tile_pool(name="ps_o", bufs=2, space="PSUM") as ps_o,
        ):
          # constant-across-reps scaffolding: loaded on rep 0 only.
          wall = big.tile([128, 2, DT, 3 * F], FP8)
          at8 = big.tile([128, NT, N], FP8)
          wout = big.tile([128, FT, DIM], F32R)
          vaug = big.tile([128, NT, FT, 2, 128], BF16)
          dinvp = big.tile([128, NT], F32)
          dinvb = big.tile([128, N], F32)
          # double-buffered x fp8 triplet: rep n prefetches rep n+1's input
          xsets = [big.tile([128, 3, DT, N], FP8, name=f"xall{i}")
                   for i in range(2)]
          for _rep in range(reps):
            xall = xsets[_rep % 2]
            xb, x8, xr8 = xall[:, 0], xall[:, 1], xall[:, 2]
            qkT = big.tile([128, 2 * FT, N], BF16)     # [f, n] f=q(0:512),k(512:1024), 16x
            v8s = big.tile([128, NT, F], FP8)          # e4m3(16*dinv_j*v)
            r8 = big.tile([128, NT, F], FP8)           # e4m3(16*dinv_j*v - v8s)
            yT = big.tile([128, FT, N], F32R)          # 16*(A_hat V)^T then merged
            yE = big.tile([128, FT, N], F32R)          # 16*normalized exp-attention

            exps = tc.alloc_tile_pool(name="exps", bufs=exps_bufs)
            small = tc.alloc_tile_pool(name="small", bufs=small_bufs)
            outs = tc.alloc_tile_pool(name="outs", bufs=5)

            # ---- input DMAs (consumption order on the sync ring) --------
            def load_x_parts(s0, s1, c0, c1):
                # x parts s0:s1 (0=x8, 1=xr8, 2=xb), columns c0:c1
                nc.sync.dma_start(
                    out=xall[:, s0:s1, :, c0:c1],
                    in_=xp_d[s0 * DIM:s1 * DIM, c0:c1].rearrange(
                        "(s t p) n -> p s t n", p=128, t=DT),
                )

            def load_w_cols(c0, c1):
                # both w parts (w8, wr8), columns c0:c1
                nc.sync.dma_start(
                    out=wall[:, :, :, c0:c1],
                    in_=wp_d[:, c0:c1].rearrange(
                        "(s t p) f -> p s t f", p=128, t=DT),
                )

            def load_w_half(s, c0, c1):
                nc.sync.dma_start(
                    out=wall[:, s, :, c0:c1],
                    in_=wp_d[s * DIM:(s + 1) * DIM, c0:c1].rearrange(
                        "(t p) f -> p t f", p=128),
                )

            def load_x_piece(s, dp, c0, c1):
                nc.sync.dma_start(
                    out=xall[:, s, 2 * dp:2 * dp + 2, c0:c1],
                    in_=xp_d[s * DIM + dp * 256:s * DIM + (dp + 1) * 256,
                             c0:c1].rearrange(
                        "(t p) n -> p t n", p=128),
                )

            if _rep == 0:
                # startup-critical prefix, finest-grained first: the very
                # first matmul needs only w8-ft0 + xb-c0-dp0 (192KB)
                load_w_half(0, 0, 128)               # w8 ft0
                load_x_parts(0, 1, 0, 512)           # xb chunk 0
                load_w_half(1, 0, 128)               # wr8 ft0
                load_x_parts(1, 3, 0, 512)           # x8+xr8 chunk 0
                load_w_cols(512, 640)                # w8+wr8 ft4
                # pair-0 attention starts here; the rest feeds the filler
                load_x_parts(1, 3, 512, 1024)        # x8+xr8 chunk 1
                # xb c1 = 16*x8 c1, an exact exponent shift: derive on ACT
                # (idle until the exp stream starts) instead of DMA
                for dp in range(DP):
                    nc.scalar.activation(
                        out=xb[:, 2 * dp:2 * dp + 2, 512:1024],
                        in_=x8[:, 2 * dp:2 * dp + 2, 512:1024],
                        func=mybir.ActivationFunctionType.Copy, scale=16.0)
                load_w_cols(2 * F, 3 * F)            # v region
                nc.sync.dma_start(out=dinvp, in_=dinvp_d[:, :])
                for t in range(1, 4):               # pair-major q/k tiles
                    load_w_cols(t * 128, (t + 1) * 128)
                    load_w_cols(512 + t * 128, 512 + (t + 1) * 128)
                nc.sync.dma_start(
                    out=wout,
                    in_=wout_d[:, :].rearrange("(t p) n -> p t n", p=128),
                )
                nc.sync.dma_start(
                    out=at8,
                    in_=at8_d[:, :].rearrange("(t p) n -> p t n", p=128),
                )
                nc.sync.dma_start(out=dinvb,
                                  in_=dinvr_d[0:1, :].to_broadcast((128, N)))
            if _rep < reps - 1:
                nc.sync.dma_start(
                    out=xsets[(_rep + 1) % 2],
                    in_=xp_d[:, :].rearrange("(s t p) n -> p s t n",
                                             p=128, t=DT))

            # vaug ones-columns (see v2 notes); junk elsewhere is zeroed once
            if _rep == 0:
                if zero_vaug:
                    nc.gpsimd.memset(vaug.bitcast(F32), 0.0)
                nc.vector.memset(vaug[:, :, :, 0, 64:65], 1.0)
                nc.vector.memset(vaug[:, :, :, 0, 96:97], 1.0)
                nc.vector.memset(vaug[:, :, :, 1, 0:1], 1.0)
                nc.vector.memset(vaug[:, :, :, 1, 32:33], 1.0)
                # preload the Exp activation table off the critical path (the
                # first real exp otherwise pays the 1.3us table load at the
                # head of the pair-0 ACT stream)
                tiny = small.tile([1, 2], F32, tag="tiny")
                nc.vector.memset(tiny, 0.0)
                nc.scalar.activation(out=tiny, in_=tiny,
                                     func=mybir.ActivationFunctionType.Exp)

            # ---- builders ----------------------------------------------
            def emit_qk(ft, chunks=tuple(range(NC2))):
                # qkT[:, ft, :] = one 128-row f-tile of (16q)^T or (16k)^T.
                # Three fp8 DoubleRow passes accumulate 16*qkv in PSUM:
                # (16x8).w8 + x8.wr8 + xr8.w8  (pass order = DMA order)
                fsl = slice(ft * 128, (ft + 1) * 128)
                for c in chunks:
                    csl = slice(c * 512, (c + 1) * 512)
                    ps = ps_mm.tile([128, 512], F32, tag="mm")
                    seq = [(wall[:, 0], xb), (wall[:, 1], x8), (wall[:, 0], xr8)]
                    i = 0
                    for wt, xt in seq:
                        for dp in range(DP):
                            nc.tensor.matmul(
                                ps,
                                wt[:, 2 * dp:2 * dp + 2, fsl],
                                xt[:, 2 * dp:2 * dp + 2, csl],
                                start=(i == 0),
                                stop=(i == 3 * DP - 1),
                                perf_mode=DR,
                            )
                            i += 1
                            yield
                    nc.vector.tensor_copy(out=qkT[:, ft, csl], in_=ps)

            def emit_v():
                # v tiles: 16v in PSUM via the same three DR passes, then
                # vaug (bf16, attnv stationary), v8s (ACT, 16*dinv*v fp8),
                # vp16 (Pool, f32), r8 (DVE residual).
                for nt in range(NT):
                    nsl = slice(nt * 128, (nt + 1) * 128)
                    ps = ps_mm.tile([128, 512], F32, tag="mm")
                    seq = [(xb, wall[:, 0]), (x8, wall[:, 1]), (xr8, wall[:, 0])]
                    i = 0
                    for xt, wt in seq:
                        for dp in range(DP):
                            nc.tensor.matmul(
                                ps,
                                xt[:, 2 * dp:2 * dp + 2, nsl],
                                wt[:, 2 * dp:2 * dp + 2, 2 * F:3 * F],
                                start=(i == 0),
                                stop=(i == 3 * DP - 1),
                                perf_mode=DR,
                            )
                            i += 1
                            yield
                    # ps readers kept cheap/idle-engine so the ps_mm slot
                    # frees quickly: vaug copies (DVE) + one Pool mul; v8s/r8
                    # then derive from the SBUF-resident vp16 off the ps path
                    ps_r = ps.rearrange("p (a b d) -> p a b d", a=FT, b=2)
                    nc.vector.tensor_copy(out=vaug[:, nt, :, 0, 0:DH],
                                          in_=ps_r[:, :, 0, :])
                    nc.vector.tensor_copy(out=vaug[:, nt, :, 1, DH:128],
                                          in_=ps_r[:, :, 1, :])
                    vp = small.tile([128, 512], F32, tag="vp")
                    nc.scalar.activation(out=vp, in_=ps,
                                         func=mybir.ActivationFunctionType.Copy,
                                         scale=dinvp[:, nt:nt + 1])
                    nc.gpsimd.tensor_copy(out=v8s[:, nt, :], in_=vp)
                    nc.gpsimd.tensor_sub(r8[:, nt, :], vp, v8s[:, nt, :])

            def ahat_unit(ft, c):
                # yT[:, ft, c] = 16*dinv_i*(A(dinv_j v))^T tile: 8 DR matmuls
                # (v8s then r8 against the shared binary A^T), dinv_i rides
                # the PSUM->SBUF copy as a broadcast-row tensor_mul.
                fsl = slice(ft * 128, (ft + 1) * 128)
                csl = slice(c * 512, (c + 1) * 512)
                ps = ps_mm.tile([128, 512], F32, tag="mm")
                i = 0
                for vt in (v8s, r8):
                    for j in range(NT // 2):
                        nc.tensor.matmul(
                            ps,
                            vt[:, 2 * j:2 * j + 2, fsl],
                            at8[:, 2 * j:2 * j + 2, csl],
                            start=(i == 0),
                            stop=(i == NT - 1),
                            perf_mode=DR,
                        )
                        i += 1
                        yield
                nc.vector.tensor_mul(yT[:, ft, csl], ps, dinvb[:, csl])

            def out_proj(nt, pool=None, split_ft3=False, copy_act=False):
                # w_out already carries the global /16; b_out added on host.
                if pool is ps_s:
                    ps2 = ps_s.tile([128, 2, 512], F32, tag="ps", name="ps2")
                    ps = ps2[:, 0, :]
                elif pool is ps_o:
                    ps = ps_o.tile([128, 512], F32, tag="po")
                else:
                    ps = ps_mm.tile([128, 512], F32, tag="mm")
                srcs = [(yT, ft) for ft in range(FT)]
                if split_ft3:
                    srcs[FT - 1:] = [(yT, FT - 1), (yE, FT - 1)]
                for i, (ysrc, ft) in enumerate(srcs):
                    nc.tensor.matmul(
                        ps,
                        ysrc[:, ft, nt * 128:(nt + 1) * 128],
                        wout[:, ft, :],
                        start=(i == 0),
                        stop=(i == len(srcs) - 1),
                    )
                    yield
                ot = outs.tile([128, DIM], F32, tag="ot")
                if pool is None and not copy_act:
                    nc.vector.tensor_copy(out=ot, in_=ps)
                else:
                    nc.scalar.activation(out=ot, in_=ps,
                                         func=mybir.ActivationFunctionType.Copy)
                nc.sync.dma_start(out=out_d[nt * 128:(nt + 1) * 128, :], in_=ot)

            def merge(ft, c):
                sl = slice(c * 512, (c + 1) * 512)
                nc.vector.tensor_add(yT[:, ft, sl], yT[:, ft, sl], yE[:, ft, sl])

            class Fill:
                """One stream of filler PE work, pulled one matmul at a time."""

                def __init__(self, gens):
                    self.gens = list(gens)

                def pull(self, n):
                    while self.gens and n > 0:
                        try:
                            next(self.gens[0])
                            n -= 1
                        except StopIteration:
                            self.gens.pop(0)

                def drain(self):
                    for g in self.gens:
                        for _ in g:
                            pass
                    self.gens = []

            def attn_pair(ht, c, fill, pre_pulls=0, intra=1, intra2=None,
                          serial=False):
                """Both heads of pair ht (parity 0/1), one 512-wide i-chunk,
                interleaved so the ACT exp stream always consumes a score
                batch produced two batches earlier. serial=True runs parity 0
                fully before parity 1 (1-batch lookahead) so the two DVE
                tails don't queue behind each other at the very end, and
                emits the ft3 merge in per-parity halves right after each
                tail (used for the last pair only)."""
                n_jb = NT // 2
                po = [ps_o.tile([128, 512], F32, tag="po", name=f"po{u}")
                      for u in range(2)]
                ets = [[None] * n_jb, [None] * n_jb]

                def scores(par, jb):
                    hb = par * 64
                    ps_sc = ps_s.tile([128, 2, 512], F32, tag="ps")
                    for e in range(2):
                        jt = jb * 2 + e
                        nc.tensor.matmul(
                            ps_sc[:, e, :],
                            qkT[hb:hb + 64, FT + ht, jt * 128:(jt + 1) * 128],
                            qkT[hb:hb + 64, ht, c * 512:(c + 1) * 512],
                        )
                    et = exps.tile([128, 2, 512], BF16, tag="exp")
                    # scores are (16q).(16k) -> fold 1/256 into the exp scale
                    nc.scalar.activation(out=et, in_=ps_sc,
                                         func=mybir.ActivationFunctionType.Exp,
                                         scale=float(SCALE / 16384.0))
                    ets[par][jb] = et

                def attnv(par, jb):
                    for e in range(2):
                        jt = jb * 2 + e
                        nc.tensor.matmul(
                            po[par],
                            vaug[:, jt, ht, par, :],
                            ets[par][jb][:, e, :],
                            start=(jt == 0),
                            stop=(jt == NT - 1),
                        )

                def tail(par):
                    hb = par * 64
                    dr_ = 64 - hb
                    rt = small.tile([128, 512], F32, tag="rt")
                    bc = small.tile([128, 512], F32, tag="bc")
                    if serial:
                        # last pair: nothing contends for the po slot, read
                        # PSUM directly to keep the endgame chain short
                        src = po[par]
                    else:
                        # evacuate po via the idle Pool engine so the ps_o
                        # slot frees ~1.3us earlier for the next pair's
                        # attnv accumulation
                        pc = small.tile([128, 512], F32, tag="pc")
                        nc.gpsimd.tensor_copy(out=pc, in_=po[par])
                        src = pc
                    nc.vector.reciprocal(out=rt[dr_:dr_ + 64, :],
                                         in_=src[dr_:dr_ + 64, :])
                    nc.vector.stream_shuffle(out=bc[hb:hb + 64, :],
                                             in_=rt[dr_:dr_ + 64, :],
                                             mask=[0] * 32)
                    nc.vector.tensor_mul(yE[hb:hb + 64, ht, c * 512:(c + 1) * 512],
                                         src[hb:hb + 64, :], bc[hb:hb + 64, :])

                if serial:
                    for par in range(2):
                        hb = par * 64
                        dr_ = 64 - hb
                        pulls = intra if par == 0 else (intra2 or intra)
                        scores(par, 0)
                        scores(par, 1)
                        for jb in range(1, n_jb):
                            fill.pull(pulls)
                            attnv(par, jb - 1)
                            if jb + 1 < n_jb:
                                scores(par, jb + 1)
                        fill.pull(pulls)
                        attnv(par, n_jb - 1)
                        # 256-wide tail+merge slices straight from PSUM so
                        # the first final out tiles unblock half a tail early
                        for hf in range(2):
                            sl = slice(c * 512 + hf * 256,
                                       c * 512 + (hf + 1) * 256)
                            psl = slice(hf * 256, (hf + 1) * 256)
                            rt = small.tile([128, 256], F32, tag="rt")
                            bc = small.tile([128, 256], F32, tag="bc")
                            nc.vector.reciprocal(out=rt[dr_:dr_ + 64, :],
                                                 in_=po[par][dr_:dr_ + 64, psl])
                            nc.vector.stream_shuffle(out=bc[hb:hb + 64, :],
                                                     in_=rt[dr_:dr_ + 64, :],
                                                     mask=[0] * 32)
                            nc.vector.tensor_mul(yE[hb:hb + 64, ht, sl],
                                                 po[par][hb:hb + 64, psl],
                                                 bc[hb:hb + 64, :])
                    return
                scores(0, 0)
                scores(1, 0)
                fill.pull(pre_pulls)
                for jb in range(1, n_jb):
                    scores(0, jb)
                    fill.pull(intra)
                    attnv(0, jb - 1)
                    scores(1, jb)
                    fill.pull(intra)
                    attnv(1, jb - 1)
                attnv(0, n_jb - 1)
                attnv(1, n_jb - 1)
                tail(0)
                tail(1)

            def attn_stream(c, hts, fill, sched):
                """Software-pipelined attention over pairs `hts` of i-chunk
                c: the score/exp stream runs a constant two half-batches
                ahead of attn@v, ACROSS pair boundaries, so neither the ACT
                pipeline latency nor a pair transition ever stalls the PE.
                sched[i] = filler matmuls to pull after unit i's scores."""
                LAG = 2
                units = [(ht, jb, par)
                         for ht in hts for jb in range(NT // 2)
                         for par in (0, 1)]
                po = {}
                ets = {}

                def scores(ht, jb, par):
                    hb = par * 64
                    ps_sc = ps_s.tile([128, 2, 512], F32, tag="ps")
                    for e in range(2):
                        jt = jb * 2 + e
                        nc.tensor.matmul(
                            ps_sc[:, e, :],
                            qkT[hb:hb + 64, FT + ht, jt * 128:(jt + 1) * 128],
                            qkT[hb:hb + 64, ht, c * 512:(c + 1) * 512],
                        )
                    et = exps.tile([128, 2, 512], BF16, tag="exp")
                    nc.scalar.activation(out=et, in_=ps_sc,
                                         func=mybir.ActivationFunctionType.Exp,
                                         scale=float(SCALE / 16384.0))
                    ets[(ht, jb, par)] = et

                def attnv(ht, jb, par):
                    if (ht, par) not in po:
                        po[(ht, par)] = ps_o.tile([128, 512], F32, tag="po",
                                                  name=f"po{ht}{par}")
                    et = ets.pop((ht, jb, par))
                    for e in range(2):
                        jt = jb * 2 + e
                        nc.tensor.matmul(
                            po[(ht, par)],
                            vaug[:, jt, ht, par, :],
                            et[:, e, :],
                            start=(jt == 0),
                            stop=(jt == NT - 1),
                        )
                    if jb == NT // 2 - 1:
                        tail(ht, par)

                def tail(ht, par):
                    hb = par * 64
                    dr_ = 64 - hb
                    rt = small.tile([128, 512], F32, tag="rt")
                    bc = small.tile([128, 512], F32, tag="bc")
                    pc = small.tile([128, 512], F32, tag="pc")
                    nc.vector.tensor_copy(out=pc, in_=po.pop((ht, par)))
                    nc.vector.reciprocal(out=rt[dr_:dr_ + 64, :],
                                         in_=pc[dr_:dr_ + 64, :])
                    nc.vector.stream_shuffle(out=bc[hb:hb + 64, :],
                                             in_=rt[dr_:dr_ + 64, :],
                                             mask=[0] * 32)
                    nc.gpsimd.tensor_mul(yE[hb:hb + 64, ht, c * 512:(c + 1) * 512],
                                         pc[hb:hb + 64, :], bc[hb:hb + 64, :])

                for i, u in enumerate(units):
                    scores(*u)
                    fill.pull(sched[i] if i < len(sched) else 0)
                    if i >= LAG:
                        attnv(*units[i - LAG])
                for i in range(LAG, 0, -1):
                    attnv(*units[len(units) - i])

            # ---- emission ----------------------------------------------
            def run(gen):
                for _ in gen:
                    pass

            # pair-0 needs only the chunk-0 q/k tiles (its first two score
            # batches use j-tiles 0-3); the k chunk-1 tile, q chunk-1, the v
            # projection and the remaining q/k tiles all stream in as filler
            # paced so each is ready just before its consumer
            run(emit_qk(0, (0,)))   # q heads 0/1, i-chunk 0
            run(emit_qk(4, (0,)))   # k heads 0/1, j-chunk 0

            fill = Fill([
                emit_qk(4, (1,)),
                emit_qk(0, (1,)),
                emit_v(),
                emit_qk(1), emit_qk(5),    # q/k heads 2/3
                emit_qk(2), emit_qk(6),    # q/k heads 4/5
                emit_qk(3), emit_qk(7),    # q/k heads 6/7
            ])
            # pull schedule: cumulative targets derived from the fill-stream
            # yield positions (emit_v epilogues gate pair-0's attnvs, each
            # qk tile's DVE copy gates the pair that consumes it)
            sched0 = [10, 10, 10, 10, 10, 10, 10, 9,
                      2, 2, 2, 2, 5, 5, 4, 4,
                      2, 2, 2, 2, 5, 5, 4, 4,
                      1, 1, 0, 0, 0, 0, 0, 0]
            attn_stream(0, range(FT), fill, sched0)
            fill.drain()

            # chunk-1 attention; filler: ahat units with merge hooks, then
            # chunk-0 out-projections
            def gmerge(ft, c):
                merge(ft, c)
                return
                yield

            fill = Fill([
                ahat_unit(0, 0), ahat_unit(1, 0), gmerge(0, 0), gmerge(1, 0),
                ahat_unit(2, 0), ahat_unit(3, 0), gmerge(2, 0), gmerge(3, 0),
                ahat_unit(0, 1), ahat_unit(1, 1),
                out_proj(0), out_proj(1), gmerge(1, 1),
                out_proj(2), out_proj(3),
            ])
            sched1 = [3] * 16 + [2] * 8
            attn_stream(1, range(3), fill, sched1)
            merge(0, 1)
            fill.drain()

            # endgame: the last pair runs parity-serial; its filler stream
            # carries the remaining ahat units (so the ft3 merge halves can
            # fire right after each tail) and then the ft0-2 partials of the
            # four final out tiles. After the pair, each final tile needs
            # only one merged-ft3 matmul, an ACT copy and its DMA.
            fps = {}

            def final_partials(nt, pool):
                if pool is ps_s:
                    ps2 = ps_s.tile([128, 2, 512], F32, tag="ps", name="ps2")
                    fp = ps2[:, 0, :]
                elif pool is ps_o:
                    fp = ps_o.tile([128, 512], F32, tag="po", name=f"fp{nt}")
                else:
                    fp = ps_mm.tile([128, 512], F32, tag="mm", name=f"fp{nt}")
                fps[nt] = fp
                for ft in range(FT):
                    nc.tensor.matmul(
                        fps[nt],
                        yT[:, ft, nt * 128:(nt + 1) * 128],
                        wout[:, ft, :],
                        start=(ft == 0),
                        stop=False,
                    )
                    yield

            def final_finish(nt):
                nc.tensor.matmul(
                    fps[nt],
                    yE[:, FT - 1, nt * 128:(nt + 1) * 128],
                    wout[:, FT - 1, :],
                    start=False,
                    stop=True,
                )
                ot = outs.tile([128, DIM], F32, tag="ot")
                # alternate ACT/DVE so the four final copies pipeline 2-wide
                if nt % 2 == 0:
                    nc.scalar.activation(out=ot, in_=fps[nt],
                                         func=mybir.ActivationFunctionType.Copy)
                else:
                    nc.vector.tensor_copy(out=ot, in_=fps[nt])
                nc.sync.dma_start(out=out_d[nt * 128:(nt + 1) * 128, :], in_=ot)

            fill2 = Fill([
                ahat_unit(2, 1), gmerge(2, 1), ahat_unit(3, 1),
                final_partials(4, None), final_partials(5, None),
                final_partials(6, ps_o), final_partials(7, ps_s),
            ])
            attn_pair(3, 1, fill2, intra=4, intra2=2, serial=True)
            fill2.drain()
            for nt in range(4, NT):
                final_finish(nt)

            outs.release()
            small.release()
            exps.release()

    nc.compile()
    return nc


def _get_program():
    global _PROGRAM
    if _PROGRAM is None:
        _PROGRAM = _build_program()
    return _PROGRAM


E4NP = ml_dtypes.float8_e4m3


def _flush(q):
    """Zero e4m3 denormals: they trigger a large PE slow path on HW (~1.5x
    on affected streams). With w prescaled 8x on the host, the flush costs
    no accuracy (residual terms recover every flushed base value)."""
    f = q.astype(np.float32)
    f[np.abs(f) < 2.0 ** -6] = 0.0
    return np.asarray(f, dtype=E4NP)


def _fp8_pair(a):
    """a ~= a8 + ar8/16 with both parts e4m3."""
    a8 = _flush(np.asarray(a, dtype=E4NP))
    ar8 = _flush(np.asarray((a - a8.astype(np.float32)) * 16.0, dtype=E4NP))
    return a8, ar8


def stage_inputs(x, adj, w_qkv, w_out):
    """Host-side staging shared by kernel() and the CoreSim harness."""
    # binary A (exact in fp8) and dinv, replicated
    A = np.ceil(adj) + np.eye(N, dtype=np.float32)
    dinv = A.sum(axis=1) ** -0.5
    at8 = np.ascontiguousarray(A.T).astype(E4NP)
    dinvp = np.ascontiguousarray(
        (dinv / 8.0).reshape(NT, 128).T).astype(np.float32)
    dinvr = (8.0 * dinv).reshape(1, N).astype(np.float32)

    # w prescaled 8x: keeps all fp8 parts (and their residuals) in e4m3's
    # normal range; the global data scale becomes 128x, folded into dinvp,
    # dinvb, the exp scale and w_out
    w8, wr8 = _fp8_pair(8.0 * w_qkv)
    wpack = np.concatenate([w8, wr8], axis=0)
    wout16 = (w_out * (1.0 / 128.0)).astype(np.float32)

    in_maps = []
    for b in range(B):
        xT = np.ascontiguousarray(x[b].T)
        x8, xr8 = _fp8_pair(xT)
        xb = np.asarray(x8.astype(np.float32) * 16.0, dtype=E4NP)  # exact shift
        in_maps.append({
            "xp": np.concatenate([xb, x8, xr8], axis=0),
            "wp": wpack,
            "at8": at8,
            "wout": wout16,
            "dinvp": dinvp,
            "dinvr": dinvr,
        })
    return in_maps


def kernel(x, adj, w_qkv, w_out, b_out):
    x = np.asarray(x, dtype=np.float32)
    adj = np.asarray(adj, dtype=np.float32)
    w_qkv = np.asarray(w_qkv, dtype=np.float32)
    w_out = np.ascontiguousarray(np.asarray(w_out, dtype=np.float32))
    b_out = np.asarray(b_out, dtype=np.float32).reshape(1, DIM)

    nc = _get_program()
    in_maps = stage_inputs(x, adj, w_qkv, w_out)
    global _last_in_maps
    _last_in_maps = in_maps
    res = run_bass_kernel_spmd(nc, in_maps, list(range(B)))
    out = np.stack([res.results[b]["out"] for b in range(B)], axis=0)
    return (out + b_out.reshape(1, 1, DIM)).astype(np.float32)


if __name__ == "__main__":
    rng = np.random.default_rng(0)
    x = rng.standard_normal((B, N, DIM), dtype=np.float32)
    adj = (rng.random((N, N), dtype=np.float32) < 0.05).astype(np.float32) * 0.5
    w_qkv = rng.standard_normal((DIM, 3 * F), dtype=np.float32) * DIM ** -0.5
    w_out = rng.standard_normal((F, DIM), dtype=np.float32) * F ** -0.5
    b_out = np.zeros(DIM, dtype=np.float32)
    out = kernel(x=x, adj=adj, w_qkv=w_qkv, w_out=w_out, b_out=b_out)
    print("out", out.shape, out.dtype, np.abs(out).max())
